# revision 1
# baseline (speedup 1.0000x reference)
"""Trainium2 Bass kernel for nn_BiLSTM_CRF_18098992185950 (8 NeuronCores).

Math reformulation (validated against the jax reference):

  conv(2ch,k3,p1) + Linear(D->1) collapse into fixed 256-d projection vectors:
      dot(l, conv1ch(x, w)) = dot(g, x),  g[d] = w0*l[d+1] + w1*l[d] + w2*l[d-1]
  so per-candidate scores are dots with 4 fixed vectors packed as G (256, 4):
      b = E[id].g_e1 (emit, cand), u = E[id].g_t0 (trans prev),
      v = E[id].g_t1 (trans cur),  a = obs_t.g_e0 (emit, obs)
  emit[t,k] = sigmoid(a_t + b_tk + ce);  trans = sigmoid(u + v + ct)

  The CRF forward DP in normal space is a matrix-product chain:
      Z = 1^T (prod_{t=0}^{1022} A_t) exp(emit_{1023}),
      A_t[j,k] = exp(sigmoid(u_t[j] + v_{t+1}[k] + ct) + emit_t[j])
  Products are associative -> 32 subchains of 32 leaves (1023 real + one
  identity pad), 4 subchains per core; the host combines 32 64x64 matrices in
  f64. Each device matmul keeps Q = (prod A)^T via matmul(lhsT=A, rhs=Q),
  rescaled by 1/s (s estimated host-side) to stay in f32 range.

Two launches: P1 streams V-sharded embedding rows and computes proj = E @ G
on the PE (memory-bound: 102 MB table read once across 8 cores); the host
gathers proj[candidate_ids] (pure indexing, ~1 MB); P2 builds the leaf
matrices (PE outer-add + ACT sigmoid/exp) and runs the matmul subchains.
"""

import numpy as np

T = 1024
K = 64
D = 256
V = 100000
NCORES = 8
NT = 128
NSUB = 8
LSUB = 16
VSH = 12544            # V-shard rows per core (98 * 128), 8*12544 >= V
NVT = VSH // 128       # 98 stream tiles
NTK = NT * K           # 8192

_PROG = {}


def _gvec(w3, l):
    g = np.zeros_like(l)
    g += w3[1] * l
    g[:-1] += w3[0] * l[1:]
    g[1:] += w3[2] * l[:-1]
    return g


def _mods():
    import concourse.bacc as bacc
    import concourse.mybir as mybir
    from concourse import tile
    return bacc, mybir, tile


def _build_p1():
    if "p1" in _PROG:
        return _PROG["p1"]
    bacc, mybir, tile = _mods()
    f32 = mybir.dt.float32

    nc = bacc.Bacc("TRN2", target_bir_lowering=False, debug=False,
                   enable_asserts=False, num_devices=NCORES)
    embs = nc.dram_tensor("embs", (VSH, D), f32, kind="ExternalInput").ap()
    gmat = nc.dram_tensor("gmat", (D, 4), f32, kind="ExternalInput").ap()
    ident = nc.dram_tensor("ident", (128, 128), f32, kind="ExternalInput").ap()
    projout = nc.dram_tensor("projout", (4, VSH), f32, kind="ExternalOutput").ap()

    with tile.TileContext(nc) as tc:
        with (
            tc.tile_pool(name="persist", bufs=1) as pp,
            tc.tile_pool(name="load", bufs=5) as lp,
            tc.tile_pool(name="stage", bufs=6) as sp,
            tc.tile_pool(name="out", bufs=3) as op,
            tc.tile_pool(name="ps_tr", bufs=4, space="PSUM") as ps_tr,
            tc.tile_pool(name="ps_pj", bufs=2, space="PSUM") as ps_pj,
        ):
            g_sb = pp.tile([128, 2, 4], f32, tag="gmat")
            nc.sync.dma_start(g_sb[:], gmat.rearrange("(c p) g -> p c g", p=128))
            id_sb = pp.tile([128, 128], f32, tag="ident")
            nc.sync.dma_start(id_sb[:], ident)

            for blk in range((NVT + 3) // 4):  # one 512KB DMA + one psum per blk
                ilo, ihi = blk * 4, min(blk * 4 + 4, NVT)
                nt = ihi - ilo
                row4 = lp.tile([128, 4, D], f32, tag="row4")
                nc.sync.dma_start(
                    row4[:, :nt, :],
                    embs[ilo * 128 : ihi * 128, :].rearrange(
                        "(t p) d -> p t d", p=128
                    ),
                )
                pj = ps_pj.tile([4, 512], f32, tag="pj")
                for i in range(ilo, ihi):
                    for ch in range(2):
                        tp = ps_tr.tile([128, 128], f32, tag="tr")
                        nc.tensor.transpose(
                            out=tp[:],
                            in_=row4[:, i - ilo, ch * 128 : (ch + 1) * 128],
                            identity=id_sb[:],
                        )
                        etT = sp.tile([128, 128], f32, tag="etT")
                        if (i + ch) % 2 == 0:
                            nc.vector.tensor_copy(out=etT[:], in_=tp[:])
                        else:
                            nc.scalar.copy(out=etT[:], in_=tp[:])
                        nc.tensor.matmul(
                            out=pj[:, (i - ilo) * 128 : (i - ilo + 1) * 128],
                            lhsT=g_sb[:, ch, :], rhs=etT[:],
                            start=(ch == 0), stop=(ch == 1),
                        )
                w = nt * 128
                pj_sb = op.tile([4, 512], f32, tag="pj_sb")
                nc.vector.tensor_copy(out=pj_sb[:, :w], in_=pj[:, :w])
                nc.sync.dma_start(
                    out=projout[:, ilo * 128 : ihi * 128], in_=pj_sb[:, :w]
                )
    nc.compile()
    _PROG["p1"] = nc
    return nc


def _build_p2():
    if "p2" in _PROG:
        return _PROG["p2"]
    bacc, mybir, tile = _mods()
    f32 = mybir.dt.float32
    AF = mybir.ActivationFunctionType
    OP = mybir.AluOpType

    nc = bacc.Bacc("TRN2", target_bir_lowering=False, debug=False,
                   enable_asserts=False, num_devices=NCORES)
    u2in = nc.dram_tensor("u2in", (2, NTK), f32, kind="ExternalInput").ap()
    v2in = nc.dram_tensor("v2in", (2, NTK), f32, kind="ExternalInput").ap()
    bt2in = nc.dram_tensor("bt2in", (NT, K), f32, kind="ExternalInput").ap()
    obs = nc.dram_tensor("obs", (NT, D), f32, kind="ExternalInput").ap()
    gmat = nc.dram_tensor("gmat", (D, 4), f32, kind="ExternalInput").ap()
    ident = nc.dram_tensor("ident", (128, 128), f32, kind="ExternalInput").ap()
    cvec = nc.dram_tensor("cvec", (1, 8), f32, kind="ExternalInput").ap()
    addend = nc.dram_tensor("addend", (K, K), f32, kind="ExternalInput").ap()
    qinit = nc.dram_tensor("qinit", (K, NSUB * K), f32, kind="ExternalInput").ap()
    qout = nc.dram_tensor("qout", (NSUB * K, K), f32, kind="ExternalOutput").ap()
    emitout = nc.dram_tensor("emitout", (K, NT), f32, kind="ExternalOutput").ap()

    with tile.TileContext(nc) as tc:
        with (
            tc.tile_pool(name="persist", bufs=1) as pp,
            tc.tile_pool(name="stage", bufs=4) as sp,
            tc.tile_pool(name="sig", bufs=3) as gp,
            tc.tile_pool(name="ps_tr", bufs=2, space="PSUM") as ps_tr,
            tc.tile_pool(name="ps_leaf", bufs=2, space="PSUM") as ps_leaf,
            tc.tile_pool(name="ps_q", bufs=4, space="PSUM") as ps_q,
        ):
            u2 = pp.tile([2, NTK], f32, tag="u2")
            nc.sync.dma_start(u2[:], u2in)
            v2 = pp.tile([2, NTK], f32, tag="v2")
            nc.sync.dma_start(v2[:], v2in)
            bt2 = pp.tile([NT, K], f32, tag="bt2")
            nc.sync.dma_start(bt2[:], bt2in)
            obs_sb = pp.tile([NT, D], f32, tag="obs")
            nc.sync.dma_start(obs_sb[:], obs)
            g_sb = pp.tile([128, 2, 4], f32, tag="gmat")
            nc.sync.dma_start(g_sb[:], gmat.rearrange("(c p) g -> p c g", p=128))
            id_sb = pp.tile([128, 128], f32, tag="ident")
            nc.sync.dma_start(id_sb[:], ident)
            add_sb = pp.tile([K, K], f32, tag="addend")
            nc.sync.dma_start(add_sb[:], addend)
            ct_col = pp.tile([K, 1], f32, tag="ct")
            nc.sync.dma_start(ct_col[:], cvec[0:1, 1:2].to_broadcast((K, 1)))
            ce_col = pp.tile([128, 1], f32, tag="ce")
            nc.sync.dma_start(ce_col[:], cvec[0:1, 2:3].to_broadcast((128, 1)))
            mask_col = pp.tile([K, 1], f32, tag="mask")
            nc.sync.dma_start(mask_col[:], cvec[0:1, 3:4].to_broadcast((K, 1)))
            mlogs_col = pp.tile([K, 1], f32, tag="mlogs")
            nc.sync.dma_start(mlogs_col[:], cvec[0:1, 4:5].to_broadcast((K, 1)))

            # a-column: obs @ g_e0 + ce
            acol_ps = ps_leaf.tile([128, 1], f32, tag="pl")
            for ch in range(2):
                tp = ps_tr.tile([128, 128], f32, tag="tr")
                nc.tensor.transpose(
                    out=tp[:], in_=obs_sb[:, ch * 128 : (ch + 1) * 128],
                    identity=id_sb[:],
                )
                obsT = sp.tile([128, 128], f32, tag="obsT")
                nc.vector.tensor_copy(out=obsT[:], in_=tp[:])
                nc.tensor.matmul(
                    out=acol_ps[:], lhsT=obsT[:], rhs=g_sb[:, ch, 3:4],
                    start=(ch == 0), stop=(ch == 1),
                )
            acol = pp.tile([128, 1], f32, tag="acol_sb")
            nc.scalar.activation(acol[:], acol_ps[:], AF.Identity, bias=ce_col[:])

            # emit columns
            emit_t = pp.tile([NT, K], f32, tag="emit_t")
            nc.scalar.activation(emit_t[:], bt2[:], AF.Sigmoid, bias=acol[:])
            etr = ps_tr.tile([K, NT], f32, tag="tr")
            nc.tensor.transpose(out=etr[:], in_=emit_t[:], identity=id_sb[:])
            emitc = pp.tile([K, NT], f32, tag="emitc")
            nc.vector.tensor_copy(out=emitc[:], in_=etr[:])
            nc.sync.dma_start(out=emitout, in_=emitc[:])

            # leaves in two passes so ACT loads the sigmoid and exp tables
            # once each instead of thrashing between them per block
            leafbuf = pp.tile([K, NT * K], f32, tag="leafbuf")
            stage2 = pp.tile([K, NT * K], f32, tag="stage2")
            for ib in range(NT // 8):
                pl = ps_leaf.tile([K, 512], f32, tag="pl")
                for q in range(8):
                    i = ib * 8 + q
                    nc.tensor.matmul(
                        out=pl[:, q * K : (q + 1) * K],
                        lhsT=u2[:, i * K : (i + 1) * K],
                        rhs=v2[:, i * K : (i + 1) * K],
                        start=True, stop=True,
                    )
                sig = gp.tile([K, 512], f32, tag="sig")
                nc.scalar.activation(sig[:], pl[:], AF.Sigmoid, bias=ct_col[:])
                nc.vector.scalar_tensor_tensor(
                    out=stage2[:, ib * 512 : (ib + 1) * 512].rearrange(
                        "p (t k) -> p t k", k=K),
                    in0=sig[:].rearrange("p (t k) -> p t k", k=K),
                    scalar=mlogs_col[:],
                    in1=emitc[:, ib * 8 : (ib + 1) * 8].unsqueeze(2).to_broadcast(
                        (K, 8, K)
                    ),
                    op0=OP.add, op1=OP.add,
                )
            for ib in range(NT // 8):
                nc.scalar.activation(
                    leafbuf[:, ib * 512 : (ib + 1) * 512],
                    stage2[:, ib * 512 : (ib + 1) * 512],
                    AF.Exp,
                )

            last = leafbuf[:, (NT - 1) * K : NT * K]
            nc.vector.scalar_tensor_tensor(
                out=last, in0=last, scalar=mask_col[:], in1=add_sb[:],
                op0=OP.mult, op1=OP.add,
            )

            # batched chain rounds: all NSUB subchains advance one leaf per
            # round; one psum bank + one DVE copy per round (leaves carry 1/s)
            qbig = pp.tile([K, NSUB * K], f32, tag="qbig")
            nc.sync.dma_start(qbig[:], qinit)
            for i in range(LSUB):
                pq = ps_q.tile([K, NSUB * K], f32, tag="pq")
                for sc in range(NSUB):
                    t = sc * LSUB + i
                    nc.tensor.matmul(
                        out=pq[:, sc * K : (sc + 1) * K],
                        lhsT=leafbuf[:, t * K : (t + 1) * K],
                        rhs=qbig[:, sc * K : (sc + 1) * K],
                        start=True, stop=True,
                    )
                nc.vector.tensor_copy(out=qbig[:], in_=pq[:])
            nc.sync.dma_start(
                out=qout.rearrange("(s j) k -> j s k", s=NSUB),
                in_=qbig[:].rearrange("p (s k) -> p s k", k=K),
            )
    nc.compile()
    _PROG["p2"] = nc
    return nc


def _host_consts(inputs):
    E = np.ascontiguousarray(np.asarray(inputs["word_embeds"], dtype=np.float32))
    ids = np.asarray(inputs["candidate_ids"]).astype(np.int64)
    obs = np.ascontiguousarray(np.asarray(inputs["observed_feats"], dtype=np.float32))

    lw_e = np.asarray(inputs["emit_lin_w"], dtype=np.float64)[0]
    lw_t = np.asarray(inputs["trans_lin_w"], dtype=np.float64)[0]
    cw_e = np.asarray(inputs["emit_conv_w"], dtype=np.float64)
    cw_t = np.asarray(inputs["trans_conv_w"], dtype=np.float64)
    g_e0 = _gvec(cw_e[0, 0], lw_e)
    g_e1 = _gvec(cw_e[0, 1], lw_e)
    g_t0 = _gvec(cw_t[0, 0], lw_t)
    g_t1 = _gvec(cw_t[0, 1], lw_t)
    ce = float(np.asarray(inputs["emit_conv_b"], np.float64)[0] * lw_e.sum()
               + np.asarray(inputs["emit_lin_b"], np.float64)[0])
    ct = float(np.asarray(inputs["trans_conv_b"], np.float64)[0] * lw_t.sum()
               + np.asarray(inputs["trans_lin_b"], np.float64)[0])
    gmat = np.stack([g_e1, g_t0, g_t1, g_e0], axis=1).astype(np.float32)

    samp = E[ids[:8].ravel()].astype(np.float64)
    sig = 1.0 / (1.0 + np.exp(-((samp @ g_t0).mean() + (samp @ g_t1).mean() + ct)))
    a8 = obs[:8].astype(np.float64) @ g_e0
    em = 1.0 / (1.0 + np.exp(-(a8.mean() + (samp @ g_e1).mean() + ce)))
    s = float(64.0 * np.exp(sig + em))
    return E, ids, obs, gmat, ce, ct, s


def _run_launches(inputs, run_kw1=None, run_kw2=None):
    """Run both launches; returns (answer, res1, res2)."""
    from concourse.bass_utils import run_bass_kernel_spmd

    run_kw1 = run_kw1 or {}
    run_kw2 = run_kw2 or {}
    E, ids, obs, gmat, ce, ct, s = _host_consts(inputs)
    ident = np.eye(128, dtype=np.float32)

    # ---- launch 1: proj = E @ G, V-sharded ----
    p1 = _build_p1()
    Epad = np.zeros((NCORES * VSH, D), dtype=np.float32)
    Epad[:V] = E
    in1 = [{"embs": Epad[c * VSH : (c + 1) * VSH], "gmat": gmat, "ident": ident}
           for c in range(NCORES)]
    res1 = run_bass_kernel_spmd(p1, in1, core_ids=list(range(NCORES)), **run_kw1)
    proj = np.concatenate([res1.results[c]["projout"] for c in range(NCORES)],
                          axis=1)[:, :V]                     # (4, V)

    # ---- host gather + staging (indexing glue only) ----
    ids_pad = np.zeros((T + 1, K), dtype=np.int64)
    ids_pad[:T] = ids
    b_g = proj[0][ids_pad]     # (1025, 64)
    u_g = proj[1][ids_pad]
    v_g = proj[2][ids_pad]

    p2 = _build_p2()
    eye64 = np.eye(K, dtype=np.float32)
    zeros64 = np.zeros((K, K), dtype=np.float32)
    in2 = []
    for c in range(NCORES):
        ta = c * NT
        u2 = np.ones((2, NTK), dtype=np.float32)
        u2[0] = u_g[ta : ta + NT].ravel()
        v2 = np.ones((2, NTK), dtype=np.float32)
        v2[1] = v_g[ta + 1 : ta + NT + 1].ravel()
        cv = np.zeros((1, 8), dtype=np.float32)
        cv[0, 0] = np.float32(1.0 / s)
        cv[0, 1] = np.float32(ct)
        cv[0, 2] = np.float32(ce)
        cv[0, 3] = 0.0 if c == NCORES - 1 else 1.0
        cv[0, 4] = np.float32(-np.log(s))
        in2.append({
            "u2in": u2,
            "v2in": v2,
            "bt2in": np.ascontiguousarray(b_g[ta : ta + NT].astype(np.float32)),
            "obs": np.ascontiguousarray(obs[ta : ta + NT]),
            "gmat": gmat,
            "ident": ident,
            "cvec": cv,
            "addend": (eye64 / np.float32(s)) if c == NCORES - 1 else zeros64,
            "qinit": np.ascontiguousarray(np.tile(eye64, (1, NSUB))),
        })
    res2 = run_bass_kernel_spmd(p2, in2, core_ids=list(range(NCORES)), **run_kw2)

    # ---- host combine in f64 ----
    P = np.eye(K, dtype=np.float64)
    acc = 0.0
    for c in range(NCORES):
        qo = res2.results[c]["qout"].astype(np.float64)
        for sc in range(NSUB):
            P = P @ qo[sc * K : (sc + 1) * K, :].T
            m = np.abs(P).max()
            P /= m
            acc += np.log(m)
    emit_last = res2.results[NCORES - 1]["emitout"][:, NT - 1].astype(np.float64)
    z = P.sum(axis=0) @ np.exp(emit_last)
    ans = np.log(z) + acc + NSUB * LSUB * NCORES * np.log(np.float64(s))
    return np.array([ans], dtype=np.float32), res1, res2


def kernel(**inputs):
    ans, _, _ = _run_launches(inputs)
    return ans


def profiled_run(inputs):
    """Run both launches with NTFF tracing; return summed exec ns (or None)."""
    import sys as _sys
    import types as _types
    try:
        if "antenv.axon_hooks" not in _sys.modules:
            from trn_agent_boot.trn_boot import _ntff_profile_via_ctypes
            hook = _ntff_profile_via_ctypes("/opt/axon/libaxon_pjrt.so")
            mod = _types.ModuleType("antenv.axon_hooks")
            mod.get_axon_ntff_profile_hook = lambda: hook
            mod.set_axon_ntff_profile_hook = lambda h: None
            _sys.modules["antenv.axon_hooks"] = mod
            import antenv
            antenv.axon_hooks = mod
    except Exception as e:
        print(f"profile shim unavailable: {e}")
        return None
    kw = {"trace": True, "trace_cores": [0]}
    ans, res1, res2 = _run_launches(inputs, run_kw1=dict(kw), run_kw2=dict(kw))
    print("profiled answer:", ans)
    for name, r in (("P1", res1), ("P2", res2)):
        tr = r.instructions_and_trace
        print(f"{name}: exec_time_ns={r.exec_time_ns}"
              + (f" trace={tr[1]}" if tr else ""))
    if res1.exec_time_ns is None or res2.exec_time_ns is None:
        return None
    return res1.exec_time_ns + res2.exec_time_ns

